# revision 3
# baseline (speedup 1.0000x reference)
"""AVWGCN (adaptive vertex-wise graph conv) Trainium2 kernel.

Math (reference):
  e  = LayerNorm(node_embeddings)                      [N, D]
  S  = softmax(elu(e @ e.T), axis=0)                   [N, N]
  supports = [I, S, 2*S@S - I]
  W  = einsum('nd,dkio->nkio', e, weights_pool)        [N, K, Din, Dout]
  b  = e @ bias_pool                                   [N, Dout]
  x_g = einsum('knm,bmc->bnkc', supports, x)           [B, N, K, Din]
  out = einsum('bnki,nkio->bno', x_g, W) + b           [B, N, Dout]

Key algebraic simplification: never materialize S@S. Use
  x_g1 = S @ x;  x_g2 = 2*S@(S@x) - x.

Sharding: nodes (rows of S) are sharded across 8 cores, 512 each.
Column-softmax denominators need a tiny (16KB) AllReduce; x_g1 needs a
1MiB-per-core AllGather so every core can form S @ x_g1.
"""

import numpy as np

N = 4096
D = 16          # embed
DIN = 32
DOUT = 64
CHEB_K = 3
B = 16
NCORES = 8
NBLK = N // NCORES          # 512 nodes per core
BC = B * DIN                # 512
MT = N // 128               # 32 m tiles
LN_EPS = 1e-12

_CACHE = {}


def _build_program():
    import concourse.bass as bass
    import concourse.bacc as bacc
    import concourse.mybir as mybir
    import concourse.tile as tile
    from contextlib import ExitStack

    f32 = mybir.dt.float32
    f32r = mybir.dt.float32r
    AF = mybir.ActivationFunctionType
    ALU = mybir.AluOpType

    nc = bacc.Bacc(
        "TRN2", target_bir_lowering=False, debug=False, num_devices=NCORES
    )

    x_full = nc.dram_tensor("x_full", [B, N, DIN], f32r, kind="ExternalInput")
    x_blk_d = nc.dram_tensor("x_blk", [B, NBLK, DIN], f32, kind="ExternalInput")
    ne_full = nc.dram_tensor("ne_full", [N, D], f32, kind="ExternalInput")
    ne_blk_d = nc.dram_tensor("ne_blk", [NBLK, D], f32, kind="ExternalInput")
    wp_d = nc.dram_tensor("wp", [D, CHEB_K * DIN * DOUT], f32r, kind="ExternalInput")
    bp_d = nc.dram_tensor("bp", [D, DOUT], f32r, kind="ExternalInput")
    gam_d = nc.dram_tensor("gam", [D], f32, kind="ExternalInput")
    bet_d = nc.dram_tensor("bet", [D], f32, kind="ExternalInput")
    ident_d = nc.dram_tensor("ident", [128, 128], f32, kind="ExternalInput")
    out_d = nc.dram_tensor("out_blk", [B, NBLK, DOUT], f32, kind="ExternalOutput")
    dbg_xg1 = nc.dram_tensor("dbg_xg1", [NBLK, BC], f32, kind="ExternalOutput")
    dbg_xg2 = nc.dram_tensor("dbg_xg2", [NBLK, BC], f32, kind="ExternalOutput")
    dbg_cs = nc.dram_tensor("dbg_cs", [128, MT], f32, kind="ExternalOutput")
    dbg_bias = nc.dram_tensor("dbg_bias", [DOUT, NBLK], f32, kind="ExternalOutput")
    dbg_et = nc.dram_tensor("dbg_et", [128, NBLK], f32, kind="ExternalOutput")
    dbg_xgT = nc.dram_tensor("dbg_xgT", [CHEB_K * DIN, 16 * 512], f32, kind="ExternalOutput")
    dbg_wt = nc.dram_tensor("dbg_wt", [CHEB_K * DIN + 1, 256 * DOUT], f32, kind="ExternalOutput")

    # internal DRAM for collectives
    cs_in = nc.dram_tensor("cs_in", [N], f32)
    cs_out = nc.dram_tensor("cs_out", [N], f32, addr_space="Shared")
    ag_in = nc.dram_tensor("ag_in", [NBLK, BC], f32)
    ag_out = nc.dram_tensor("ag_out", [N, BC], f32, addr_space="Shared")

    rg = [list(range(NCORES))]

    with tile.TileContext(nc) as tc, ExitStack() as ctx:
        persist = ctx.enter_context(tc.tile_pool(name="persist", bufs=1))
        work = ctx.enter_context(tc.tile_pool(name="work", bufs=3))
        psA = ctx.enter_context(tc.tile_pool(name="psA", bufs=2, space="PSUM"))
        psB = ctx.enter_context(tc.tile_pool(name="psB", bufs=2, space="PSUM"))

        # ---------------- persistent loads ----------------
        ident = persist.tile([128, 128], f32, tag="ident")
        nc.sync.dma_start(ident[:], ident_d[:])
        bp_sb = persist.tile([D, DOUT], f32r, tag="bp")
        nc.sync.dma_start(bp_sb[:], bp_d[:])
        eps_sb = persist.tile([128, 1], f32, tag="eps")
        nc.vector.memset(eps_sb[:], LN_EPS)
        gam_sb = persist.tile([128, D], f32, tag="gam")
        nc.sync.dma_start(gam_sb[:], gam_d[:].unsqueeze(0).broadcast_to([128, D]))
        bet_sb = persist.tile([128, D], f32, tag="bet")
        nc.sync.dma_start(bet_sb[:], bet_d[:].unsqueeze(0).broadcast_to([128, D]))

        # core's own x block, [n, (b,c)]: 4 tiles (persist: used in phase C too)
        xblk_re = x_blk_d.rearrange("b (t p) c -> t p b c", p=128)
        x_blk = []
        for t in range(NBLK // 128):
            xt = persist.tile([128, B, DIN], f32, tag=f"xblk{t}")
            nc.sync.dma_start(xt[:], xblk_re[t])
            x_blk.append(xt)

        cs_part = persist.tile([128, MT], f32, tag="cs_part")
        rcol = persist.tile([128, MT], f32, tag="rcol")
        ebT = persist.tile([D, NBLK], f32r, tag="ebT")
        xg1_blk = [persist.tile([128, BC], f32, tag=f"xg1_{j}", name=f"xg1_{j}") for j in range(4)]
        xg2_blk = [persist.tile([128, BC], f32, tag=f"xg2_{j}", name=f"xg2_{j}") for j in range(4)]

        # ne loaded once: [128, (t, d)]
        ne_sb = persist.tile([128, MT, D], f32, tag="ne_sb")
        nc.sync.dma_start(ne_sb[:], ne_full.rearrange("(t p) d -> p t d", p=128))
        neb_sb = persist.tile([128, NBLK // 128, D], f32, tag="neb_sb")
        nc.sync.dma_start(neb_sb[:], ne_blk_d.rearrange("(t p) d -> p t d", p=128))

        # ---------------- layernorm helper ----------------
        def layernorm_tile(raw, dst):
            mu = work.tile([128, 1], f32, tag="ln_mu")
            nc.vector.reduce_sum(mu[:], raw, axis=mybir.AxisListType.X)
            nc.vector.tensor_scalar_mul(mu[:], mu[:], 1.0 / D)
            cen = work.tile([128, D], f32, tag="ln_cen")
            nc.vector.tensor_scalar_sub(cen[:], raw, mu[:])
            sq = work.tile([128, D], f32, tag="ln_sq")
            ssq = work.tile([128, 1], f32, tag="ln_ssq")
            nc.vector.scalar_tensor_tensor(
                sq[:], cen[:], 1.0, cen[:], ALU.mult, ALU.mult, accum_out=ssq[:]
            )
            sd = work.tile([128, 1], f32, tag="ln_sd")
            nc.scalar.activation(sd[:], ssq[:], AF.Sqrt, bias=eps_sb[:], scale=1.0 / D)
            rstd = work.tile([128, 1], f32, tag="ln_rstd")
            nc.vector.reciprocal(rstd[:], sd[:])
            e1 = work.tile([128, D], f32, tag="ln_e1")
            nc.vector.tensor_scalar_mul(e1[:], cen[:], rstd[:])
            nc.vector.scalar_tensor_tensor(
                e1[:], e1[:], 1.0, gam_sb[:], ALU.mult, ALU.mult
            )
            nc.vector.tensor_tensor(dst, e1[:], bet_sb[:], ALU.add)

        # ================= PHASE A/B: softmax + propagation =================
        with tc.tile_pool(name="phA", bufs=1) as phA:
            # layernorm block; transpose to eb^T
            for t in range(NBLK // 128):
                et = work.tile([128, D], f32, tag="e_cur")
                layernorm_tile(neb_sb[:, t, :], et[:])
                pt = psB.tile([128, 512], f32, tag="sm")
                nc.tensor.transpose(pt[0:D, 0:128], et[:], ident[:])
                nc.vector.tensor_copy(ebT[:, t * 128 : (t + 1) * 128], pt[0:D, 0:128])

            # x in [m, (b,c)] layout: 32 tiles of [128, 512]
            x_re = x_full.rearrange("b (t p) c -> t p b c", p=128)
            x_sb = []
            for t in range(MT):
                xt = phA.tile([128, B, DIN], f32r, tag=f"xs{t}")
                nc.sync.dma_start(xt[:], x_re[t])
                x_sb.append(xt)

            # E^T tiles + colsum partials
            et_tiles = []
            for t in range(MT):
                et = work.tile([128, D], f32, tag="e_cur")
                layernorm_tile(ne_sb[:, t, :], et[:])
                pt = psB.tile([128, 512], f32, tag="sm")
                nc.tensor.transpose(pt[0:D, 0:128], et[:], ident[:])
                eTt = work.tile([D, 128], f32r, tag="eTt")
                nc.vector.tensor_copy(eTt[:], pt[0:D, 0:128])
                gps = psA.tile([128, NBLK], f32, tag="big")
                nc.tensor.matmul(
                    gps[:],
                    eTt[:],
                    ebT[:],
                    start=True,
                    stop=True,
                )
                r = work.tile([128, NBLK], f32, tag="elu_r", bufs=2)
                nc.scalar.activation(r[:], gps[:], AF.Relu)
                m = work.tile([128, NBLK], f32, tag="elu_m", bufs=2)
                nc.vector.scalar_tensor_tensor(
                    m[:], gps[:], 1.0, r[:], ALU.mult, ALU.subtract
                )
                tt = work.tile([128, NBLK], f32, tag="elu_t", bufs=2)
                nc.scalar.activation(tt[:], m[:], AF.Exp)
                s = work.tile([128, NBLK], f32, tag="elu_s", bufs=2)
                nc.vector.scalar_tensor_tensor(
                    s[:], r[:], -1.0, tt[:], ALU.add, ALU.add
                )
                et_t = phA.tile([128, NBLK], f32r, tag=f"ET{t}")
                nc.scalar.activation(
                    et_t[:], s[:], AF.Exp, accum_out=cs_part[:, t : t + 1]
                )
                et_tiles.append(et_t)

            # colsum AllReduce -> rcol = 1/colsum
            cs_re = cs_in.rearrange("(t p) -> t p", p=128)
            for t in range(MT):
                nc.sync.dma_start(cs_re[t].unsqueeze(-1), cs_part[:, t : t + 1])
            nc.gpsimd.collective_compute(
                "AllReduce", mybir.AluOpType.add, replica_groups=rg,
                ins=[cs_in[:]], outs=[cs_out[:]],
            )
            cso_re = cs_out.rearrange("(t p) -> t p", p=128)
            for t in range(MT):
                nc.sync.dma_start(rcol[:, t : t + 1], cso_re[t].unsqueeze(-1))
            nc.vector.reciprocal(rcol[:], rcol[:])

            # normalize: S^T = E^T * rcol[m]  (per-partition), round to f32r
            for t in range(MT):
                nc.vector.tensor_scalar_mul(
                    et_tiles[t][:], et_tiles[t][:], rcol[:, t : t + 1]
                )

            # matmul1: x_g1_blk = S_blk @ x
            for j in range(NBLK // 128):
                ps = psA.tile([128, BC], f32, tag="big")
                for t in range(MT):
                    nc.tensor.matmul(
                        ps[:],
                        et_tiles[t][:, j * 128 : (j + 1) * 128],
                        x_sb[t][:].rearrange("p b c -> p (b c)"),
                        start=(t == 0),
                        stop=(t == MT - 1),
                    )
                nc.vector.tensor_copy(xg1_blk[j][:], ps[:])
                nc.sync.dma_start(ag_in[j * 128 : (j + 1) * 128, :], xg1_blk[j][:])

            # AllGather x_g1 across cores
            nc.gpsimd.collective_compute(
                "AllGather", mybir.AluOpType.bypass, replica_groups=rg,
                ins=[ag_in[:]], outs=[ag_out[:]],
            )


            # reload gathered x_g1 (reuse x tiles), scale by rcol, matmul2
            ago_re = ag_out.rearrange("(t p) f -> t p f", p=128)
            for t in range(MT):
                nc.gpsimd.dma_start(
                    x_sb[t][:], ago_re[t].rearrange("p (b c) -> p b c", b=B)
                )

            for j in range(NBLK // 128):
                ps = psA.tile([128, BC], f32, tag="big")
                for t in range(MT):
                    nc.tensor.matmul(
                        ps[:],
                        et_tiles[t][:, j * 128 : (j + 1) * 128],
                        x_sb[t][:].rearrange("p b c -> p (b c)"),
                        start=(t == 0),
                        stop=(t == MT - 1),
                    )
                # x_g2 = 2*ps - x_blk
                nc.vector.scalar_tensor_tensor(
                    xg2_blk[j][:],
                    ps[:],
                    2.0,
                    x_blk[j][:].rearrange("p b c -> p (b c)"),
                    ALU.mult,
                    ALU.subtract,
                )
                nc.sync.dma_start(dbg_xg2[j * 128 : (j + 1) * 128, :], xg2_blk[j][:])
                nc.sync.dma_start(dbg_xg1[j * 128 : (j + 1) * 128, :], xg1_blk[j][:])
            nc.sync.dma_start(dbg_cs[:], cs_part[:])
            nc.sync.dma_start(dbg_et[:], et_tiles[0][:].bitcast(f32))

        # ================= PHASE C: per-node conv =================
        with tc.tile_pool(name="phC", bufs=1) as phC, \
             tc.tile_pool(name="wt", bufs=1) as wt_pool:
            # x_g^T via DVE stream transpose:
            # xgT[32k+c, 512r + 32b + n_in] = x_g[b, 32r+n_in, k, c]
            xgT = phC.tile([CHEB_K * DIN + 1, 16 * 512], f32, tag="xgT")
            nc.vector.memset(xgT[CHEB_K * DIN : CHEB_K * DIN + 1, :], 1.0)
            wp_sb = phC.tile([D, CHEB_K * DIN * DOUT], f32r, tag="wp")
            nc.sync.dma_start(wp_sb[:], wp_d[:])

            def stream_tr(src_tiles, k):
                # out free layout (n_in, b): stream order is (b, n_in), so
                # scatter via strided out AP [[1,16],[16,32]]
                for r in range(16):
                    t4, s4 = divmod(r, 4)
                    ap = src_tiles[t4][:]
                    if len(ap.shape) == 3:
                        ap = ap.rearrange("p b c -> p (b c)")
                    dst = xgT[32 * k : 32 * k + 32, r * 512 : (r + 1) * 512]
                    dst = dst.rearrange("p (n b) -> p b n", n=32, b=B)
                    nc.vector.transpose(dst, ap[32 * s4 : 32 * s4 + 32, :])

            stream_tr(x_blk, 0)
            stream_tr(xg1_blk, 1)
            stream_tr(xg2_blk, 2)
            nc.sync.dma_start(dbg_xgT[:], xgT[0 : CHEB_K * DIN, :])

            wp_koi = wp_sb[:].rearrange("d (k i o) -> d k i o", k=CHEB_K, i=DIN)

            for ch in range(2):
                # W^T chunk [96, 256*64]: lhsT = wp[:, (k,i), o], rhs = ebT
                wt = wt_pool.tile([CHEB_K * DIN + 1, 256 * DOUT], f32, tag="wt")
                wt_no = wt[0 : CHEB_K * DIN, :].rearrange("p (n o) -> p n o", o=DOUT)
                # bias row: bias_nat [128n, 64o] tiles -> flatten into row 96
                for q in range(2):
                    bn_ps = psB.tile([128, 512], f32, tag="sm")
                    nc.tensor.matmul(
                        bn_ps[0:128, 0:DOUT],
                        ebT[:, ch * 256 + q * 128 : ch * 256 + (q + 1) * 128],
                        bp_sb[:],
                        start=True,
                        stop=True,
                    )
                    bn_sb = work.tile([128, DOUT], f32, tag="bn_sb")
                    nc.vector.tensor_copy(bn_sb[:], bn_ps[0:128, 0:DOUT])
                    nc.sync.dma_start(
                        wt[CHEB_K * DIN : CHEB_K * DIN + 1,
                           q * 128 * DOUT : (q + 1) * 128 * DOUT],
                        bn_sb[:],
                    )
                for o in range(DOUT):
                    wps = psB.tile([128, 512], f32, tag="sm")
                    nc.tensor.matmul(
                        wps[0 : CHEB_K * DIN, 0:256],
                        wp_koi[:, :, :, o],
                        ebT[:, ch * 256 : (ch + 1) * 256],
                        start=True,
                        stop=True,
                    )
                    nc.vector.tensor_copy(
                        wt_no[:, :, o], wps[0 : CHEB_K * DIN, 0:256]
                    )

                if ch == 0:
                    nc.sync.dma_start(dbg_wt[:], wt[:])
                for grp in range(8):  # 32-node groups within chunk
                    cps = psB.tile([128, 512], f32, tag="sm")
                    for s in range(32):
                        nloc = grp * 32 + s          # node within chunk
                        n = ch * 256 + nloc          # node within block
                        r, n_in = divmod(n, 32)
                        nc.tensor.matmul(
                            cps[0:DOUT, s * B : (s + 1) * B],
                            wt[:, nloc * DOUT : (nloc + 1) * DOUT],
                            xgT[:, n * B : (n + 1) * B],
                            start=True,
                            stop=True,
                        )
                    n0 = ch * 256 + grp * 32
                    oT = work.tile([DOUT, 32 * B], f32, tag="oT")
                    nc.vector.tensor_copy(oT[:], cps[0:DOUT, :])
                    # transpose [o, (n,b)] -> [(n8,b16), o] and store, 4x
                    for q in range(4):
                        fps = psB.tile([128, 512], f32, tag="sm")
                        nc.tensor.transpose(
                            fps[0:128, 0:DOUT],
                            oT[:, q * 128 : (q + 1) * 128],
                            ident[0:DOUT, 0:DOUT],
                        )
                        of = work.tile([128, DOUT], f32, tag="outF")
                        nc.vector.tensor_copy(of[:], fps[0:128, 0:DOUT])
                        g8 = n0 + q * 8
                        nc.sync.dma_start(
                            out_d.rearrange("b n o -> n b o")[g8 : g8 + 8, :, :],
                            of[:],
                        )


    nc.compile()
    return nc


def _get_program():
    if "nc" not in _CACHE:
        _CACHE["nc"] = _build_program()
    return _CACHE["nc"]


def _prepare_in_maps(x, node_embeddings, weights_pool, bias_pool, ln_gamma, ln_beta):
    x = np.ascontiguousarray(np.asarray(x, dtype=np.float32))
    ne = np.ascontiguousarray(np.asarray(node_embeddings, dtype=np.float32))
    wp = np.ascontiguousarray(
        np.asarray(weights_pool, dtype=np.float32).reshape(D, -1)
    )
    bp = np.ascontiguousarray(np.asarray(bias_pool, dtype=np.float32))
    gam = np.ascontiguousarray(np.asarray(ln_gamma, dtype=np.float32))
    bet = np.ascontiguousarray(np.asarray(ln_beta, dtype=np.float32))
    ident = np.eye(128, dtype=np.float32)

    in_maps = []
    for c in range(NCORES):
        sl = slice(c * NBLK, (c + 1) * NBLK)
        in_maps.append(
            {
                "x_full": x,
                "x_blk": np.ascontiguousarray(x[:, sl, :]),
                "ne_full": ne,
                "ne_blk": np.ascontiguousarray(ne[sl, :]),
                "wp": wp,
                "bp": bp,
                "gam": gam,
                "bet": bet,
                "ident": ident,
            }
        )

    return in_maps


def kernel(x, node_embeddings, weights_pool, bias_pool, ln_gamma, ln_beta):
    from concourse.bass_utils import run_bass_kernel_spmd

    nc = _get_program()
    in_maps = _prepare_in_maps(
        x, node_embeddings, weights_pool, bias_pool, ln_gamma, ln_beta
    )
    res = run_bass_kernel_spmd(nc, in_maps, list(range(NCORES)))
    out = np.concatenate([res.results[c]["out_blk"] for c in range(NCORES)], axis=1)
    return out



# revision 8
# speedup vs baseline: 2.2266x; 2.2266x over previous
"""AVWGCN (adaptive vertex-wise graph conv) Trainium2 kernel.

Math (reference):
  e  = LayerNorm(node_embeddings)                      [N, D]
  S  = softmax(elu(e @ e.T), axis=0)                   [N, N]
  supports = [I, S, 2*S@S - I]
  W  = einsum('nd,dkio->nkio', e, weights_pool)        [N, K, Din, Dout]
  b  = e @ bias_pool                                   [N, Dout]
  x_g = einsum('knm,bmc->bnkc', supports, x)           [B, N, K, Din]
  out = einsum('bnki,nkio->bno', x_g, W) + b           [B, N, Dout]

Never materialize S@S: x_g1 = S @ x; x_g2 = 2*S@x_g1 - x.

Sharding: node rows of S across 8 cores (512 each). Column-softmax
denominators via 16KB AllReduce; x_g1 via bf16 AllGather.

exp(elu(g)) computed Exp-only (no act-table swaps):
  t = exp(g); r = max(t, exp(min(t,1) - 1)).

Per-node conv out[b,n,:] uses lhsT = x_g^T node column (free dim = b),
rhs = W^T node block, so results land as [b, (n,o)] for contiguous
output DMA rows.
"""

import numpy as np

N = 4096
D = 16          # embed
DIN = 32
DOUT = 64
CHEB_K = 3
B = 16
NCORES = 8
NBLK = N // NCORES          # 512 nodes per core
BC = B * DIN                # 512
MT = N // 128               # 32 m tiles
KI = CHEB_K * DIN           # 96
LN_EPS = 1e-12

_CACHE = {}


def _build_program():
    import concourse.bass as bass
    import concourse.bacc as bacc
    import concourse.mybir as mybir
    import concourse.tile as tile
    from contextlib import ExitStack

    f32 = mybir.dt.float32
    f32r = mybir.dt.float32r
    bf16 = mybir.dt.bfloat16
    AF = mybir.ActivationFunctionType
    ALU = mybir.AluOpType

    nc = bacc.Bacc(
        "TRN2", target_bir_lowering=False, debug=False, num_devices=NCORES
    )

    # -------- DRAM inputs (host-prepped layouts) --------
    x_t_d = nc.dram_tensor("x_t", [N, BC], bf16, kind="ExternalInput")
    xblk_t_d = nc.dram_tensor("x_blk_t", [NBLK, BC], bf16, kind="ExternalInput")
    xTb_d = nc.dram_tensor("xTb", [DIN, NBLK * B], bf16, kind="ExternalInput")
    ne_re_d = nc.dram_tensor("ne_re", [128, MT * D], f32, kind="ExternalInput")
    neb_re_d = nc.dram_tensor("neb_re", [128, (NBLK // 128) * D], f32, kind="ExternalInput")
    wpb_d = nc.dram_tensor("wpb", [D, DOUT * (KI + 1)], bf16, kind="ExternalInput")
    gam_d = nc.dram_tensor("gam", [D], f32, kind="ExternalInput")
    bet_d = nc.dram_tensor("bet", [D], f32, kind="ExternalInput")
    ident_d = nc.dram_tensor("ident", [128, 128], f32, kind="ExternalInput")
    out_d = nc.dram_tensor("out_blk", [B, NBLK, DOUT], f32, kind="ExternalOutput")

    # internal DRAM for collectives
    cs_in = nc.dram_tensor("cs_in", [128, MT], f32)
    cs_out = nc.dram_tensor("cs_out", [128, MT], f32, addr_space="Shared")
    ag_in = nc.dram_tensor("ag_in", [NBLK, BC], bf16)
    ag_out = nc.dram_tensor("ag_out", [N, BC], bf16, addr_space="Shared")

    rg = [list(range(NCORES))]

    with tile.TileContext(nc) as tc, ExitStack() as ctx:
        persist = ctx.enter_context(tc.tile_pool(name="persist", bufs=1))
        work = ctx.enter_context(tc.tile_pool(name="work", bufs=3))
        psA = ctx.enter_context(tc.tile_pool(name="psA", bufs=2, space="PSUM"))
        psB = ctx.enter_context(tc.tile_pool(name="psB", bufs=2, space="PSUM"))
        psW = ctx.enter_context(tc.tile_pool(name="psW", bufs=2, space="PSUM"))
        psC = ctx.enter_context(tc.tile_pool(name="psC", bufs=2, space="PSUM"))
        outp = ctx.enter_context(tc.tile_pool(name="outp", bufs=2))

        # ---------------- persistent loads ----------------
        ident = persist.tile([128, 128], f32, tag="ident")
        nc.sync.dma_start(ident[:], ident_d[:])
        eps_sb = persist.tile([128, 1], f32, tag="eps")
        nc.vector.memset(eps_sb[:], LN_EPS)
        neg1_sb = persist.tile([128, 1], f32, tag="neg1")
        nc.vector.memset(neg1_sb[:], -1.0)
        gam_sb = persist.tile([128, D], f32, tag="gam")
        nc.sync.dma_start(gam_sb[:], gam_d[:].unsqueeze(0).broadcast_to([128, D]))
        bet_sb = persist.tile([128, D], f32, tag="bet")
        nc.sync.dma_start(bet_sb[:], bet_d[:].unsqueeze(0).broadcast_to([128, D]))
        wpb_sb = persist.tile([D, DOUT * (KI + 1)], bf16, tag="wpb")
        nc.sync.dma_start(wpb_sb[:], wpb_d[:])
        ne_sb = persist.tile([128, MT, D], f32, tag="ne_sb")
        nc.sync.dma_start(ne_sb[:], ne_re_d[:].rearrange("p (t d) -> p t d", d=D))
        neb_sb = persist.tile([128, NBLK // 128, D], f32, tag="neb_sb")
        nc.sync.dma_start(neb_sb[:], neb_re_d[:].rearrange("p (t d) -> p t d", d=D))

        # x tiles [m-part, (b c)] bf16; reused later for gathered xg1
        xt_re = x_t_d.rearrange("(t p) f -> t p f", p=128)
        x_sb = []
        for t in range(MT):
            xt = persist.tile([128, BC], bf16, tag=f"xs{t}")
            nc.sync.dma_start(xt[:], xt_re[t])
            x_sb.append(xt)
        # own block copy (x_sb gets overwritten by the AllGather reload)
        xb_re = xblk_t_d.rearrange("(t p) f -> t p f", p=128)
        x_blk = []
        for t in range(NBLK // 128):
            xt = persist.tile([128, BC], bf16, tag=f"xblk{t}")
            nc.sync.dma_start(xt[:], xb_re[t])
            x_blk.append(xt)

        # x_g^T tile: rows (k,i) + ones row; cols = 16*node + b
        xgT = persist.tile([KI + 1, B * NBLK], bf16, tag="xgT")
        nc.sync.dma_start(xgT[0:DIN, :], xTb_d[:])
        nc.vector.memset(xgT[KI : KI + 1, :], 1.0)

        # W^T: rows (k,i)+bias; cols = 64*node + o
        wt = persist.tile([KI + 1, NBLK * DOUT], bf16, tag="wt")
        wt_no = wt[:].rearrange("p (n o) -> p n o", o=DOUT)

        ebT = persist.tile([D, NBLK], f32r, tag="ebT")
        ebT_bf = persist.tile([D, NBLK], bf16, tag="ebT_bf")
        cs_part = persist.tile([128, MT], f32, tag="cs_part")
        rcol = persist.tile([128, MT], f32, tag="rcol")
        etn = [persist.tile([128, NBLK], bf16, tag=f"etn{t}", name=f"etn{t}") for t in range(MT)]
        xg1_bf = [persist.tile([128, BC], bf16, tag=f"xg1_{j}", name=f"xg1_{j}") for j in range(4)]

        # ---------------- layernorm helper ----------------
        def layernorm_tile(raw, dst):
            mu = work.tile([128, 1], f32, tag="ln_mu")
            nc.vector.reduce_sum(mu[:], raw, axis=mybir.AxisListType.X)
            nc.vector.tensor_scalar_mul(mu[:], mu[:], 1.0 / D)
            cen = work.tile([128, D], f32, tag="ln_cen")
            nc.vector.tensor_scalar_sub(cen[:], raw, mu[:])
            sq = work.tile([128, D], f32, tag="ln_sq")
            ssq = work.tile([128, 1], f32, tag="ln_ssq")
            nc.vector.scalar_tensor_tensor(
                sq[:], cen[:], 1.0, cen[:], ALU.mult, ALU.mult, accum_out=ssq[:]
            )
            sd = work.tile([128, 1], f32, tag="ln_sd")
            nc.scalar.activation(sd[:], ssq[:], AF.Sqrt, bias=eps_sb[:], scale=1.0 / D)
            rstd = work.tile([128, 1], f32, tag="ln_rstd")
            nc.vector.reciprocal(rstd[:], sd[:])
            e1 = work.tile([128, D], f32, tag="ln_e1")
            nc.vector.tensor_scalar_mul(e1[:], cen[:], rstd[:])
            nc.vector.scalar_tensor_tensor(
                e1[:], e1[:], 1.0, gam_sb[:], ALU.mult, ALU.mult
            )
            nc.vector.tensor_tensor(dst, e1[:], bet_sb[:], ALU.add)

        # ============ PHASE 1: all layernorms first (one Sqrt table) ============
        with tc.tile_pool(name="ph1", bufs=1) as ph1:
            e_blk = ph1.tile([128, NBLK // 128, D], f32, tag="e_blk")
            for t in range(NBLK // 128):
                layernorm_tile(neb_sb[:, t, :], e_blk[:, t, :])
            e_full = ph1.tile([128, MT, D], f32, tag="e_full")
            for t in range(MT):
                layernorm_tile(ne_sb[:, t, :], e_full[:, t, :])

            # transpose own-block e -> ebT [D, NBLK]
            for t in range(NBLK // 128):
                pt = psB.tile([128, 128], f32, tag="tr")
                nc.tensor.transpose(pt[0:D, 0:128], e_blk[:, t, :], ident[:])
                nc.vector.tensor_copy(ebT[:, t * 128 : (t + 1) * 128], pt[0:D, 0:128])

            # ============ PHASE 2: scores + exp(elu) (Exp only) ============
            for t in range(MT):
                pt = psB.tile([128, 128], f32, tag="tr")
                nc.tensor.transpose(pt[0:D, 0:128], e_full[:, t, :], ident[:])
                eTt = work.tile([D, 128], f32r, tag="eTt", bufs=2)
                nc.vector.tensor_copy(eTt[:], pt[0:D, 0:128])
                gps = psA.tile([128, NBLK], f32, tag="big")
                nc.tensor.matmul(gps[:], eTt[:], ebT[:], start=True, stop=True)
                t_e = work.tile([128, NBLK], bf16, tag="elu_t", bufs=2)
                nc.scalar.activation(t_e[:], gps[:], AF.Exp)
                mn = work.tile([128, NBLK], bf16, tag="elu_m", bufs=2)
                nc.vector.tensor_scalar_min(mn[:], t_e[:], 1.0)
                v = work.tile([128, NBLK], bf16, tag="elu_v", bufs=2)
                nc.scalar.activation(v[:], mn[:], AF.Exp, bias=neg1_sb[:])
                nc.vector.scalar_tensor_tensor(
                    etn[t][:], t_e[:], 1.0, v[:], ALU.mult, ALU.max,
                    accum_out=cs_part[:, t : t + 1],
                )

            # colsum partials out; kick AllReduce
            nc.sync.dma_start(cs_in[:], cs_part[:])
            nc.gpsimd.collective_compute(
                "AllReduce", mybir.AluOpType.add, replica_groups=rg,
                ins=[cs_in[:]], outs=[cs_out[:]],
            )

            # ---- fill the AllReduce bubble: W^T formation ----
            nc.vector.tensor_copy(ebT_bf[:], ebT[:].bitcast(f32))
            wpb_v = wpb_sb[:].rearrange("p (o q) -> p o q", q=KI + 1)
            for o in range(DOUT):
                wps = psW.tile([128, NBLK], f32, tag="wps")
                nc.tensor.matmul(
                    wps[0 : KI + 1, :],
                    wpb_v[:, o, :],
                    ebT_bf[:],
                    start=True,
                    stop=True,
                )
                eng = nc.vector if (o % 2 == 0) else nc.scalar
                if eng is nc.vector:
                    nc.vector.tensor_copy(wt_no[:, :, o], wps[0 : KI + 1, :])
                else:
                    nc.scalar.activation(wt_no[:, :, o], wps[0 : KI + 1, :], AF.Copy)

            # rcol = 1/colsum (blocks on AllReduce)
            nc.sync.dma_start(rcol[:], cs_out[:])
            nc.vector.reciprocal(rcol[:], rcol[:])
            for t in range(MT):
                nc.vector.tensor_scalar_mul(etn[t][:], etn[t][:], rcol[:, t : t + 1])

            # ============ PHASE 3: x_g1 = S_blk @ x ============
            for j in range(NBLK // 128):
                ps = psA.tile([128, BC], f32, tag="big")
                for t in range(MT):
                    nc.tensor.matmul(
                        ps[:],
                        etn[t][:, j * 128 : (j + 1) * 128],
                        x_sb[t][:],
                        start=(t == 0),
                        stop=(t == MT - 1),
                    )
                nc.vector.tensor_copy(xg1_bf[j][:], ps[:])
                nc.sync.dma_start(ag_in[j * 128 : (j + 1) * 128, :], xg1_bf[j][:])

            nc.gpsimd.collective_compute(
                "AllGather", mybir.AluOpType.bypass, replica_groups=rg,
                ins=[ag_in[:]], outs=[ag_out[:]],
            )

            # ---- fill the AllGather bubble: transpose xg1 into xgT (k=1) ----
            def stream_tr(src_tiles, k):
                for r in range(B):
                    t4, s4 = divmod(r, 4)
                    dst = xgT[DIN * k : DIN * (k + 1), r * 512 : (r + 1) * 512]
                    dst = dst.rearrange("p (n b) -> p b n", n=32, b=B)
                    nc.vector.transpose(dst, src_tiles[t4][32 * s4 : 32 * s4 + 32, :])

            stream_tr(xg1_bf, 1)

            # reload gathered xg1 into x_sb (waits on AllGather)
            ago_re = ag_out.rearrange("(t p) f -> t p f", p=128)
            for t in range(MT):
                nc.sync.dma_start(x_sb[t][:], ago_re[t])

            # ============ PHASE 4: x_g2 = 2*S_blk @ xg1 - x ============
            xg2_bf = [None] * 4
            for j in range(NBLK // 128):
                ps = psA.tile([128, BC], f32, tag="big")
                for t in range(MT):
                    nc.tensor.matmul(
                        ps[:],
                        etn[t][:, j * 128 : (j + 1) * 128],
                        x_sb[t][:],
                        start=(t == 0),
                        stop=(t == MT - 1),
                    )
                xg2 = work.tile([128, BC], bf16, tag="xg2", bufs=2)
                nc.vector.scalar_tensor_tensor(
                    xg2[:], ps[:], 2.0, x_blk[j][:], ALU.mult, ALU.subtract
                )
                xg2_bf[j] = xg2
                # transpose this j-block into xgT (k=2)
                for s4 in range(4):
                    r = j * 4 + s4
                    dst = xgT[2 * DIN : 3 * DIN, r * 512 : (r + 1) * 512]
                    dst = dst.rearrange("p (n b) -> p b n", n=32, b=B)
                    nc.vector.transpose(dst, xg2[32 * s4 : 32 * s4 + 32, :])

        # ============ PHASE 5: per-node conv ============
        xgT_n = xgT[:].rearrange("p (n b) -> p n b", b=B)
        for grp in range(NBLK // 8):
            ps = psC.tile([16, 512], f32, tag="cps")
            for s in range(8):
                n = grp * 8 + s
                nc.tensor.matmul(
                    ps[0:B, s * DOUT : (s + 1) * DOUT],
                    xgT_n[:, n, :],
                    wt_no[:, n, :],
                    start=True,
                    stop=True,
                )
            q = grp % 2
            if q == 0:
                onat = outp.tile([16, 1024], f32, tag="onat")
            if grp % 2 == 0:
                nc.vector.tensor_copy(onat[:, q * 512 : (q + 1) * 512], ps[0:B, :])
            else:
                nc.scalar.activation(
                    onat[:, q * 512 : (q + 1) * 512], ps[0:B, :], AF.Copy
                )
            if q == 1:
                n0 = (grp - 1) * 8
                nc.sync.dma_start(out_d[:, n0 : n0 + 16, :], onat[:])

    nc.compile()
    return nc


def _get_program():
    if "nc" not in _CACHE:
        _CACHE["nc"] = _build_program()
    return _CACHE["nc"]


def _prepare_in_maps(x, node_embeddings, weights_pool, bias_pool, ln_gamma, ln_beta):
    import ml_dtypes

    bf16 = ml_dtypes.bfloat16
    x = np.asarray(x, dtype=np.float32)
    ne = np.asarray(node_embeddings, dtype=np.float32)
    wp = np.asarray(weights_pool, dtype=np.float32).reshape(D, CHEB_K * DIN, DOUT)
    bp = np.asarray(bias_pool, dtype=np.float32)
    gam = np.ascontiguousarray(np.asarray(ln_gamma, dtype=np.float32))
    bet = np.ascontiguousarray(np.asarray(ln_beta, dtype=np.float32))
    ident = np.eye(128, dtype=np.float32)

    # x transposed to [n, (b c)]
    xt = np.ascontiguousarray(
        x.transpose(1, 0, 2).reshape(N, BC).astype(bf16)
    )
    # ne rearranged [(p), (t d)]
    ne_re = np.ascontiguousarray(
        ne.reshape(MT, 128, D).transpose(1, 0, 2).reshape(128, MT * D)
    )
    # weights_pool + bias packed: [d, o*(KI+1) + ki], bias at ki=KI
    wpb = np.zeros((D, DOUT * (KI + 1)), dtype=np.float32)
    for o in range(DOUT):
        wpb[:, o * (KI + 1) : o * (KI + 1) + KI] = wp[:, :, o]
        wpb[:, o * (KI + 1) + KI] = bp[:, o]
    wpb = wpb.astype(bf16)

    in_maps = []
    for c in range(NCORES):
        sl = slice(c * NBLK, (c + 1) * NBLK)
        xTb = np.ascontiguousarray(
            x[:, sl, :].transpose(2, 1, 0).reshape(DIN, NBLK * B).astype(bf16)
        )
        neb_re = np.ascontiguousarray(
            ne[sl].reshape(NBLK // 128, 128, D).transpose(1, 0, 2)
            .reshape(128, (NBLK // 128) * D)
        )
        in_maps.append(
            {
                "x_t": xt,
                "x_blk_t": np.ascontiguousarray(xt[sl]),
                "xTb": xTb,
                "ne_re": ne_re,
                "neb_re": neb_re,
                "wpb": wpb,
                "gam": gam,
                "bet": bet,
                "ident": ident,
            }
        )
    return in_maps


def kernel(x, node_embeddings, weights_pool, bias_pool, ln_gamma, ln_beta):
    from concourse.bass_utils import run_bass_kernel_spmd

    nc = _get_program()
    in_maps = _prepare_in_maps(
        x, node_embeddings, weights_pool, bias_pool, ln_gamma, ln_beta
    )
    res = run_bass_kernel_spmd(nc, in_maps, list(range(NCORES)))
    out = np.concatenate([res.results[c]["out_blk"] for c in range(NCORES)], axis=1)
    return out


# revision 10
# speedup vs baseline: 2.6585x; 1.1940x over previous
"""AVWGCN (adaptive vertex-wise graph conv) Trainium2 kernel.

Math (reference):
  e  = LayerNorm(node_embeddings)                      [N, D]
  S  = softmax(elu(e @ e.T), axis=0)                   [N, N]
  supports = [I, S, 2*S@S - I]
  W  = einsum('nd,dkio->nkio', e, weights_pool)        [N, K, Din, Dout]
  b  = e @ bias_pool                                   [N, Dout]
  x_g = einsum('knm,bmc->bnkc', supports, x)           [B, N, K, Din]
  out = einsum('bnki,nkio->bno', x_g, W) + b           [B, N, Dout]

Never materialize S@S: x_g1 = S @ x; x_g2 = 2*S@x_g1 - x.

Sharding: node rows of S across 8 cores (512 each). Column-softmax
denominators via 16KB AllReduce; x_g1 via bf16 AllGather.

exp(elu(g)) computed Exp-only (no act-table swaps):
  t = exp(g); r = max(t, exp(min(t,1) - 1)).

x_g2 is computed directly in transposed layout (lhsT = gathered-xg1
column group, rhs = S^T tile), folding 2*ps - x^T against the x^T rows
already sitting in xgT.  Per-node conv: lhsT = W^T node block (o-major),
rhs = x_g^T node column, output [o, (n, b)] stored as [DOUT, NBLK, B]
in DRAM; the host transposes to [B, NBLK, DOUT] when unsharding.
"""

import numpy as np

N = 4096
D = 16          # embed
DIN = 32
DOUT = 64
CHEB_K = 3
B = 16
NCORES = 8
NBLK = N // NCORES          # 512 nodes per core
BC = B * DIN                # 512
MT = N // 128               # 32 m tiles
KI = CHEB_K * DIN           # 96
LN_EPS = 1e-12

_CACHE = {}


def _build_program():
    import concourse.bass as bass
    import concourse.bacc as bacc
    import concourse.mybir as mybir
    import concourse.tile as tile
    from contextlib import ExitStack

    f32 = mybir.dt.float32
    f32r = mybir.dt.float32r
    bf16 = mybir.dt.bfloat16
    AF = mybir.ActivationFunctionType
    ALU = mybir.AluOpType
    AX = mybir.AxisListType

    nc = bacc.Bacc(
        "TRN2", target_bir_lowering=False, debug=False, num_devices=NCORES
    )

    # -------- DRAM inputs (host-prepped layouts) --------
    x_t_d = nc.dram_tensor("x_t", [N, BC], bf16, kind="ExternalInput")
    # x^T of own block: [c, (b, n)]
    xTb_d = nc.dram_tensor("xTb", [DIN, B * NBLK], bf16, kind="ExternalInput")
    ne_re_d = nc.dram_tensor("ne_re", [128, MT * D], f32, kind="ExternalInput")
    neb_re_d = nc.dram_tensor("neb_re", [128, (NBLK // 128) * D], f32, kind="ExternalInput")
    wpb_d = nc.dram_tensor("wpb", [D, DOUT * (KI + 1)], bf16, kind="ExternalInput")
    gam_d = nc.dram_tensor("gam", [D], f32, kind="ExternalInput")
    bet_d = nc.dram_tensor("bet", [D], f32, kind="ExternalInput")
    ident_d = nc.dram_tensor("ident", [128, 128], f32, kind="ExternalInput")
    identb_d = nc.dram_tensor("identb", [128, 128], bf16, kind="ExternalInput")
    # output transposed: [o, n, b]; host flips to [b, n, o]
    out_d = nc.dram_tensor("out_blk", [DOUT, NBLK, B], f32, kind="ExternalOutput")

    # internal DRAM for collectives
    cs_in = nc.dram_tensor("cs_in", [128, MT], f32)
    cs_out = nc.dram_tensor("cs_out", [128, MT], f32, addr_space="Shared")
    ag_in = nc.dram_tensor("ag_in", [NBLK, BC], bf16)
    ag_out = nc.dram_tensor("ag_out", [N, BC], bf16, addr_space="Shared")

    rg = [list(range(NCORES))]

    with tile.TileContext(nc) as tc, ExitStack() as ctx:
        persist = ctx.enter_context(tc.tile_pool(name="persist", bufs=1))
        work = ctx.enter_context(tc.tile_pool(name="work", bufs=3))
        psA = ctx.enter_context(tc.tile_pool(name="psA", bufs=2, space="PSUM"))
        psB = ctx.enter_context(tc.tile_pool(name="psB", bufs=2, space="PSUM"))
        psBb = ctx.enter_context(tc.tile_pool(name="psBb", bufs=1, space="PSUM"))
        psC = ctx.enter_context(tc.tile_pool(name="psC", bufs=3, space="PSUM"))
        outp = ctx.enter_context(tc.tile_pool(name="outp", bufs=2))

        # ---------------- persistent loads ----------------
        ident = persist.tile([128, 128], f32, tag="ident")
        nc.sync.dma_start(ident[:], ident_d[:])
        identb = persist.tile([128, 128], bf16, tag="identb")
        nc.sync.dma_start(identb[:], identb_d[:])
        eps_sb = persist.tile([128, 1], f32, tag="eps")
        nc.vector.memset(eps_sb[:], LN_EPS)
        neg1_sb = persist.tile([128, 1], f32, tag="neg1")
        nc.vector.memset(neg1_sb[:], -1.0)
        gam_sb = persist.tile([128, D], f32, tag="gam")
        nc.sync.dma_start(gam_sb[:], gam_d[:].unsqueeze(0).broadcast_to([128, D]))
        bet_sb = persist.tile([128, D], f32, tag="bet")
        nc.sync.dma_start(bet_sb[:], bet_d[:].unsqueeze(0).broadcast_to([128, D]))
        wpb_sb = persist.tile([D, DOUT * (KI + 1)], bf16, tag="wpb")
        nc.sync.dma_start(wpb_sb[:], wpb_d[:])
        ne_sb = persist.tile([128, MT, D], f32, tag="ne_sb")
        nc.sync.dma_start(ne_sb[:], ne_re_d[:].rearrange("p (t d) -> p t d", d=D))
        neb_sb = persist.tile([128, NBLK // 128, D], f32, tag="neb_sb")
        nc.sync.dma_start(neb_sb[:], neb_re_d[:].rearrange("p (t d) -> p t d", d=D))

        # x tiles [m-part, (b c)] bf16; later overwritten with gathered xg1
        xt_re = x_t_d.rearrange("(t p) f -> t p f", p=128)
        x_sb = []
        for t in range(MT):
            xt = persist.tile([128, BC], bf16, tag=f"xs{t}")
            nc.sync.dma_start(xt[:], xt_re[t])
            x_sb.append(xt)

        # x_g^T tile: rows (k,i) + ones row; cols = 512*b + n
        xgT = persist.tile([KI + 1, B * NBLK], bf16, tag="xgT")
        nc.sync.dma_start(xgT[0:DIN, :], xTb_d[:])
        nc.vector.memset(xgT[KI : KI + 1, :], 1.0)

        # W^T: rows (k,i)+bias; cols = 512*o + n  (o-major)
        wt = persist.tile([KI + 1, DOUT * NBLK], bf16, tag="wt")
        wt_on = wt[:].rearrange("p (o n) -> p o n", n=NBLK)

        ebT = persist.tile([D, NBLK], f32r, tag="ebT")
        ebT_bf = persist.tile([D, NBLK], bf16, tag="ebT_bf")
        cs_part = persist.tile([128, MT], f32, tag="cs_part")
        rcol = persist.tile([128, MT], f32, tag="rcol")
        etn = [persist.tile([128, NBLK], bf16, tag=f"etn{t}", name=f"etn{t}") for t in range(MT)]
        xg1_bf = [persist.tile([128, BC], bf16, tag=f"xg1_{j}", name=f"xg1_{j}") for j in range(4)]

        # ---------------- batched layernorm ----------------
        def layernorm_batch(src, dst, nt):
            # src/dst: [128, nt, D]
            mu = work.tile([128, nt], f32, tag="ln_mu")
            nc.vector.tensor_reduce(mu[:], src, axis=AX.X, op=ALU.add)
            nc.vector.tensor_scalar_mul(mu[:], mu[:], 1.0 / D)
            muB = mu[:].unsqueeze(-1).broadcast_to([128, nt, D])
            cen = work.tile([128, nt, D], f32, tag="ln_cen")
            nc.vector.tensor_tensor(cen[:], src, muB, ALU.subtract)
            sq = work.tile([128, nt, D], f32, tag="ln_sq")
            nc.vector.tensor_tensor(sq[:], cen[:], cen[:], ALU.mult)
            ssq = work.tile([128, nt], f32, tag="ln_ssq")
            nc.vector.tensor_reduce(ssq[:], sq[:], axis=AX.X, op=ALU.add)
            sd = work.tile([128, nt], f32, tag="ln_sd")
            nc.scalar.activation(sd[:], ssq[:], AF.Sqrt, bias=eps_sb[:], scale=1.0 / D)
            rstd = work.tile([128, nt], f32, tag="ln_rstd")
            nc.vector.reciprocal(rstd[:], sd[:])
            rstdB = rstd[:].unsqueeze(-1).broadcast_to([128, nt, D])
            e1 = work.tile([128, nt, D], f32, tag="ln_e1")
            nc.vector.tensor_tensor(e1[:], cen[:], rstdB, ALU.mult)
            gamB = gam_sb[:].unsqueeze(1).broadcast_to([128, nt, D])
            betB = bet_sb[:].unsqueeze(1).broadcast_to([128, nt, D])
            nc.vector.tensor_tensor(e1[:], e1[:], gamB, ALU.mult)
            nc.vector.tensor_tensor(dst, e1[:], betB, ALU.add)

        # ============ PHASE 1: layernorms ============
        with tc.tile_pool(name="ph1", bufs=1) as ph1:
            e_blk = ph1.tile([128, NBLK // 128, D], f32, tag="e_blk")
            layernorm_batch(neb_sb[:], e_blk[:], NBLK // 128)
            e_full = ph1.tile([128, MT, D], f32, tag="e_full")
            layernorm_batch(ne_sb[:], e_full[:], MT)

            # transpose own-block e -> ebT [D, NBLK]
            for t in range(NBLK // 128):
                pt = psB.tile([128, 128], f32, tag="tr")
                nc.tensor.transpose(pt[0:D, 0:128], e_blk[:, t, :], ident[:])
                nc.vector.tensor_copy(ebT[:, t * 128 : (t + 1) * 128], pt[0:D, 0:128])
            nc.vector.tensor_copy(ebT_bf[:], ebT[:].bitcast(f32))

            # ============ PHASE 2: scores + exp(elu) (Exp only) ============
            for t in range(MT):
                pt = psB.tile([128, 128], f32, tag="tr")
                nc.tensor.transpose(pt[0:D, 0:128], e_full[:, t, :], ident[:])
                eTt = work.tile([D, 128], f32r, tag="eTt", bufs=3)
                nc.vector.tensor_copy(eTt[:], pt[0:D, 0:128])
                gps = psA.tile([128, NBLK], f32, tag="big")
                nc.tensor.matmul(gps[:], eTt[:], ebT[:], start=True, stop=True)
                t_e = work.tile([128, NBLK], bf16, tag="elu_t", bufs=3)
                nc.scalar.activation(t_e[:], gps[:], AF.Exp)
                mn = work.tile([128, NBLK], bf16, tag="elu_m", bufs=3)
                nc.vector.tensor_scalar_min(mn[:], t_e[:], 1.0)
                v = work.tile([128, NBLK], bf16, tag="elu_v", bufs=3)
                nc.scalar.activation(v[:], mn[:], AF.Exp, bias=neg1_sb[:])
                nc.vector.scalar_tensor_tensor(
                    etn[t][:], t_e[:], 1.0, v[:], ALU.mult, ALU.max,
                    accum_out=cs_part[:, t : t + 1],
                )

            # colsum partials out; kick AllReduce
            nc.sync.dma_start(cs_in[:], cs_part[:])
            nc.gpsimd.collective_compute(
                "AllReduce", mybir.AluOpType.add, replica_groups=rg,
                ins=[cs_in[:]], outs=[cs_out[:]],
            )

            # ---- fill the AllReduce bubble: W^T formation (o-major) ----
            wpb_v = wpb_sb[:].rearrange("p (o q) -> p o q", q=KI + 1)
            for o in range(DOUT):
                wps = psA.tile([128, NBLK], f32, tag="big")
                nc.tensor.matmul(
                    wps[0 : KI + 1, :], wpb_v[:, o, :], ebT_bf[:],
                    start=True, stop=True,
                )
                if o % 2 == 0:
                    nc.vector.tensor_copy(wt_on[:, o, :], wps[0 : KI + 1, :])
                else:
                    nc.scalar.activation(wt_on[:, o, :], wps[0 : KI + 1, :], AF.Copy)

            # rcol = 1/colsum (blocks on AllReduce)
            nc.sync.dma_start(rcol[:], cs_out[:])
            nc.vector.reciprocal(rcol[:], rcol[:])
            for t in range(MT):
                nc.vector.tensor_scalar_mul(etn[t][:], etn[t][:], rcol[:, t : t + 1])

            # ============ PHASE 3: x_g1 = S_blk @ x ============
            for j in range(NBLK // 128):
                ps = psA.tile([128, BC], f32, tag="big")
                for t in range(MT):
                    nc.tensor.matmul(
                        ps[:],
                        etn[t][:, j * 128 : (j + 1) * 128],
                        x_sb[t][:],
                        start=(t == 0),
                        stop=(t == MT - 1),
                    )
                nc.vector.tensor_copy(xg1_bf[j][:], ps[:])
                nc.sync.dma_start(ag_in[j * 128 : (j + 1) * 128, :], xg1_bf[j][:])

            nc.gpsimd.collective_compute(
                "AllGather", mybir.AluOpType.bypass, replica_groups=rg,
                ins=[ag_in[:]], outs=[ag_out[:]],
            )

            # ---- fill the AllGather bubble: PE-transpose xg1 into xgT (k=1)
            for j in range(NBLK // 128):
                for ch in range(4):
                    tp = psBb.tile([128, 128], bf16, tag="trb")
                    nc.tensor.transpose(
                        tp[:], xg1_bf[j][:, ch * 128 : (ch + 1) * 128], identb[:]
                    )
                    for bl in range(4):
                        b = ch * 4 + bl
                        nc.vector.tensor_copy(
                            xgT[DIN : 2 * DIN,
                                b * NBLK + j * 128 : b * NBLK + (j + 1) * 128],
                            tp[bl * 32 : bl * 32 + 32, :],
                        )

            # reload gathered xg1 into x_sb (waits on AllGather)
            ago_re = ag_out.rearrange("(t p) f -> t p f", p=128)
            for t in range(MT):
                nc.sync.dma_start(x_sb[t][:], ago_re[t])

            # ===== PHASE 4: x_g2^T = 2*(S_blk @ xg1)^T - x^T, direct =====
            for g in range(4):
                ps = psA.tile([128, BC], f32, tag="big")
                for t in range(MT):
                    nc.tensor.matmul(
                        ps[:],
                        x_sb[t][:, g * 128 : (g + 1) * 128],
                        etn[t][:],
                        start=(t == 0),
                        stop=(t == MT - 1),
                    )
                for bl in range(4):
                    b = g * 4 + bl
                    nc.vector.scalar_tensor_tensor(
                        xgT[2 * DIN : 3 * DIN, b * NBLK : (b + 1) * NBLK],
                        ps[bl * 32 : bl * 32 + 32, :],
                        2.0,
                        xgT[0:DIN, b * NBLK : (b + 1) * NBLK],
                        ALU.mult,
                        ALU.subtract,
                    )

        # ============ PHASE 5: per-node conv ============
        xgT_n = xgT[:].rearrange("p (b n) -> p n b", b=B)
        for grp in range(NBLK // 32):
            ps = psC.tile([64, 512], f32, tag="cps")
            for s in range(32):
                n = grp * 32 + s
                nc.tensor.matmul(
                    ps[0:DOUT, s * B : (s + 1) * B],
                    wt_on[:, :, n],
                    xgT_n[:, n, :],
                    start=True,
                    stop=True,
                )
            q = grp % 2
            if q == 0:
                onat = outp.tile([64, 1024], f32, tag="onat")
            nc.vector.tensor_copy(onat[:, q * 512 : (q + 1) * 512], ps[0:DOUT, :])
            if q == 1:
                n0 = (grp - 1) * 32
                nc.sync.dma_start(
                    out_d.rearrange("o n b -> o (n b)")[:, n0 * B : (n0 + 64) * B],
                    onat[:],
                )

    nc.compile()
    return nc


def _get_program():
    if "nc" not in _CACHE:
        _CACHE["nc"] = _build_program()
    return _CACHE["nc"]


def _prepare_in_maps(x, node_embeddings, weights_pool, bias_pool, ln_gamma, ln_beta):
    import ml_dtypes

    bf16 = ml_dtypes.bfloat16
    x = np.asarray(x, dtype=np.float32)
    ne = np.asarray(node_embeddings, dtype=np.float32)
    wp = np.asarray(weights_pool, dtype=np.float32).reshape(D, CHEB_K * DIN, DOUT)
    bp = np.asarray(bias_pool, dtype=np.float32)
    gam = np.ascontiguousarray(np.asarray(ln_gamma, dtype=np.float32))
    bet = np.ascontiguousarray(np.asarray(ln_beta, dtype=np.float32))
    ident = np.eye(128, dtype=np.float32)
    identb = np.eye(128, dtype=np.float32).astype(bf16)

    # x transposed to [n, (b c)]
    xt = np.ascontiguousarray(x.transpose(1, 0, 2).reshape(N, BC).astype(bf16))
    # ne rearranged [(p), (t d)]
    ne_re = np.ascontiguousarray(
        ne.reshape(MT, 128, D).transpose(1, 0, 2).reshape(128, MT * D)
    )
    # weights_pool + bias packed: [d, o*(KI+1) + ki], bias at ki=KI
    wpb = np.zeros((D, DOUT * (KI + 1)), dtype=np.float32)
    for o in range(DOUT):
        wpb[:, o * (KI + 1) : o * (KI + 1) + KI] = wp[:, :, o]
        wpb[:, o * (KI + 1) + KI] = bp[:, o]
    wpb = wpb.astype(bf16)

    in_maps = []
    for c in range(NCORES):
        sl = slice(c * NBLK, (c + 1) * NBLK)
        # x^T own block: [c, (b, n)]
        xTb = np.ascontiguousarray(
            x[:, sl, :].transpose(2, 0, 1).reshape(DIN, B * NBLK).astype(bf16)
        )
        neb_re = np.ascontiguousarray(
            ne[sl].reshape(NBLK // 128, 128, D).transpose(1, 0, 2)
            .reshape(128, (NBLK // 128) * D)
        )
        in_maps.append(
            {
                "x_t": xt,
                "xTb": xTb,
                "ne_re": ne_re,
                "neb_re": neb_re,
                "wpb": wpb,
                "gam": gam,
                "bet": bet,
                "ident": ident,
                "identb": identb,
            }
        )
    return in_maps


def kernel(x, node_embeddings, weights_pool, bias_pool, ln_gamma, ln_beta):
    from concourse.bass_utils import run_bass_kernel_spmd

    nc = _get_program()
    in_maps = _prepare_in_maps(
        x, node_embeddings, weights_pool, bias_pool, ln_gamma, ln_beta
    )
    res = run_bass_kernel_spmd(nc, in_maps, list(range(NCORES)))
    # out_blk is [DOUT, NBLK, B] per core; flip to [B, NBLK, DOUT]
    out = np.concatenate(
        [res.results[c]["out_blk"].transpose(2, 1, 0) for c in range(NCORES)],
        axis=1,
    )
    return np.ascontiguousarray(out)


# revision 12
# speedup vs baseline: 2.9052x; 1.0928x over previous
"""AVWGCN (adaptive vertex-wise graph conv) Trainium2 kernel.

Math (reference):
  e  = LayerNorm(node_embeddings)                      [N, D]
  S  = softmax(elu(e @ e.T), axis=0)                   [N, N]
  supports = [I, S, 2*S@S - I]
  W  = einsum('nd,dkio->nkio', e, weights_pool)        [N, K, Din, Dout]
  b  = e @ bias_pool                                   [N, Dout]
  x_g = einsum('knm,bmc->bnkc', supports, x)           [B, N, K, Din]
  out = einsum('bnki,nkio->bno', x_g, W) + b           [B, N, Dout]

Never materialize S@S: x_g1 = S @ x; x_g2 = 2*S@x_g1 - x.

Sharding: node rows of S across 8 cores (512 each). Column-softmax
denominators via 16KB AllReduce; x_g1 via bf16 AllGather.

exp(elu(g)) computed Exp-only (no act-table swaps):
  t = exp(g); r = max(t, exp(min(t,1) - 1)).

x_g2 is computed directly in transposed layout (lhsT = gathered-xg1
column group, rhs = S^T tile), folding 2*ps - x^T against the x^T rows
already sitting in xgT.  Per-node conv: lhsT = W^T node block (o-major),
rhs = x_g^T node column, output [o, (n, b)] stored as [DOUT, NBLK, B]
in DRAM; the host transposes to [B, NBLK, DOUT] when unsharding.
"""

import numpy as np

N = 4096
D = 16          # embed
DIN = 32
DOUT = 64
CHEB_K = 3
B = 16
NCORES = 8
NBLK = N // NCORES          # 512 nodes per core
BC = B * DIN                # 512
MT = N // 128               # 32 m tiles
KI = CHEB_K * DIN           # 96
LN_EPS = 1e-12

_CACHE = {}


def _build_program():
    import concourse.bass as bass
    import concourse.bacc as bacc
    import concourse.mybir as mybir
    import concourse.tile as tile
    from contextlib import ExitStack

    f32 = mybir.dt.float32
    f32r = mybir.dt.float32r
    bf16 = mybir.dt.bfloat16
    AF = mybir.ActivationFunctionType
    ALU = mybir.AluOpType
    AX = mybir.AxisListType

    nc = bacc.Bacc(
        "TRN2", target_bir_lowering=False, debug=False, num_devices=NCORES
    )

    # -------- DRAM inputs (host-prepped layouts) --------
    x_t_d = nc.dram_tensor("x_t", [N, BC], bf16, kind="ExternalInput")
    # x^T of own block: [c, (n, b)]
    xTb_d = nc.dram_tensor("xTb", [DIN, B * NBLK], bf16, kind="ExternalInput")
    ne_re_d = nc.dram_tensor("ne_re", [128, MT * D], f32, kind="ExternalInput")
    neb_re_d = nc.dram_tensor("neb_re", [128, (NBLK // 128) * D], f32, kind="ExternalInput")
    wpb_d = nc.dram_tensor("wpb", [D, DOUT * (KI + 1)], bf16, kind="ExternalInput")
    gam_d = nc.dram_tensor("gam", [D], f32, kind="ExternalInput")
    bet_d = nc.dram_tensor("bet", [D], f32, kind="ExternalInput")
    ident_d = nc.dram_tensor("ident", [128, 128], f32, kind="ExternalInput")
    identb_d = nc.dram_tensor("identb", [128, 128], bf16, kind="ExternalInput")
    out_d = nc.dram_tensor("out_blk", [B, NBLK, DOUT], f32, kind="ExternalOutput")

    # internal DRAM for collectives
    cs_in = nc.dram_tensor("cs_in", [128, MT], f32)
    cs_out = nc.dram_tensor("cs_out", [128, MT], f32, addr_space="Shared")
    ag_in = nc.dram_tensor("ag_in", [NBLK, BC], bf16)
    ag_out = nc.dram_tensor("ag_out", [N, BC], bf16, addr_space="Shared")

    rg = [list(range(NCORES))]

    with tile.TileContext(nc) as tc, ExitStack() as ctx:
        persist = ctx.enter_context(tc.tile_pool(name="persist", bufs=1))
        work = ctx.enter_context(tc.tile_pool(name="work", bufs=3))
        psA = ctx.enter_context(tc.tile_pool(name="psA", bufs=3, space="PSUM"))
        psB = ctx.enter_context(tc.tile_pool(name="psB", bufs=1, space="PSUM"))
        psBb = ctx.enter_context(tc.tile_pool(name="psBb", bufs=1, space="PSUM"))
        psC = ctx.enter_context(tc.tile_pool(name="psC", bufs=3, space="PSUM"))
        outp = ctx.enter_context(tc.tile_pool(name="outp", bufs=2))

        # ---------------- persistent loads ----------------
        ident = persist.tile([128, 128], f32, tag="ident")
        nc.sync.dma_start(ident[:], ident_d[:])
        identb = persist.tile([128, 128], bf16, tag="identb")
        nc.sync.dma_start(identb[:], identb_d[:])
        eps_sb = persist.tile([128, 1], f32, tag="eps")
        nc.vector.memset(eps_sb[:], LN_EPS)
        neg1_sb = persist.tile([128, 1], f32, tag="neg1")
        nc.vector.memset(neg1_sb[:], -1.0)
        gam_sb = persist.tile([128, D], f32, tag="gam")
        nc.sync.dma_start(gam_sb[:], gam_d[:].unsqueeze(0).broadcast_to([128, D]))
        bet_sb = persist.tile([128, D], f32, tag="bet")
        nc.sync.dma_start(bet_sb[:], bet_d[:].unsqueeze(0).broadcast_to([128, D]))
        wpb_sb = persist.tile([D, DOUT * (KI + 1)], bf16, tag="wpb")
        nc.sync.dma_start(wpb_sb[:], wpb_d[:])
        ne_sb = persist.tile([128, MT, D], f32, tag="ne_sb")
        nc.sync.dma_start(ne_sb[:], ne_re_d[:].rearrange("p (t d) -> p t d", d=D))
        neb_sb = persist.tile([128, NBLK // 128, D], f32, tag="neb_sb")
        nc.sync.dma_start(neb_sb[:], neb_re_d[:].rearrange("p (t d) -> p t d", d=D))

        # x_g^T tile: rows (k,i) + ones row; cols = 16*node + b
        xgT = persist.tile([KI + 1, B * NBLK], bf16, tag="xgT")
        nc.sync.dma_start(xgT[0:DIN, :], xTb_d[:])
        nc.vector.memset(xgT[KI : KI + 1, :], 1.0)

        # x tiles [m-part, (b c)] bf16; later overwritten with gathered xg1
        xt_re = x_t_d.rearrange("(t p) f -> t p f", p=128)
        x_sb = []
        for t in range(MT):
            xt = persist.tile([128, BC], bf16, tag=f"xs{t}")
            nc.sync.dma_start(xt[:], xt_re[t])
            x_sb.append(xt)

        xgT_k0 = xgT[0:DIN, :].rearrange("p (n b) -> p n b", b=B)
        xgT_k1 = xgT[DIN : 2 * DIN, :].rearrange("p (n b) -> p n b", b=B)
        xgT_k2 = xgT[2 * DIN : 3 * DIN, :].rearrange("p (n b) -> p n b", b=B)

        # W^T: rows (k,i)+bias; cols = 512*o + n  (o-major)
        wt = persist.tile([KI + 1, DOUT * NBLK], bf16, tag="wt")
        wt_on = wt[:].rearrange("p (o n) -> p o n", n=NBLK)

        ebT = persist.tile([D, NBLK], f32r, tag="ebT")
        ebT_bf = persist.tile([D, NBLK], bf16, tag="ebT_bf")
        cs_part = persist.tile([128, MT], f32, tag="cs_part")
        rcol = persist.tile([128, MT], f32, tag="rcol")
        etn = [persist.tile([128, NBLK], bf16, tag=f"etn{t}", name=f"etn{t}") for t in range(MT)]
        xg1_bf = [persist.tile([128, BC], bf16, tag=f"xg1_{j}", name=f"xg1_{j}") for j in range(4)]

        # ---------------- batched layernorm ----------------
        def layernorm_batch(src, dst, nt):
            # src/dst: [128, nt, D]
            mu = work.tile([128, nt], f32, tag="ln_mu")
            nc.vector.tensor_reduce(mu[:], src, axis=AX.X, op=ALU.add)
            nc.vector.tensor_scalar_mul(mu[:], mu[:], 1.0 / D)
            muB = mu[:].unsqueeze(-1).broadcast_to([128, nt, D])
            cen = work.tile([128, nt, D], f32, tag="ln_cen")
            nc.vector.tensor_tensor(cen[:], src, muB, ALU.subtract)
            sq = work.tile([128, nt, D], f32, tag="ln_sq")
            nc.vector.tensor_tensor(sq[:], cen[:], cen[:], ALU.mult)
            ssq = work.tile([128, nt], f32, tag="ln_ssq")
            nc.vector.tensor_reduce(ssq[:], sq[:], axis=AX.X, op=ALU.add)
            sd = work.tile([128, nt], f32, tag="ln_sd")
            nc.scalar.activation(sd[:], ssq[:], AF.Sqrt, bias=eps_sb[:], scale=1.0 / D)
            rstd = work.tile([128, nt], f32, tag="ln_rstd")
            nc.vector.reciprocal(rstd[:], sd[:])
            rstdB = rstd[:].unsqueeze(-1).broadcast_to([128, nt, D])
            e1 = work.tile([128, nt, D], f32, tag="ln_e1")
            nc.vector.tensor_tensor(e1[:], cen[:], rstdB, ALU.mult)
            gamB = gam_sb[:].unsqueeze(1).broadcast_to([128, nt, D])
            betB = bet_sb[:].unsqueeze(1).broadcast_to([128, nt, D])
            nc.vector.tensor_tensor(e1[:], e1[:], gamB, ALU.mult)
            nc.vector.tensor_tensor(dst, e1[:], betB, ALU.add)

        # ============ PHASE 1: layernorms ============
        with tc.tile_pool(name="ph1", bufs=1) as ph1:
            e_blk = ph1.tile([128, NBLK // 128, D], f32, tag="e_blk")
            layernorm_batch(neb_sb[:], e_blk[:], NBLK // 128)
            e_full = ph1.tile([128, MT, D], f32, tag="e_full")
            layernorm_batch(ne_sb[:], e_full[:], MT)

            # transpose own-block e -> ebT [D, NBLK]
            for t in range(NBLK // 128):
                pt = psB.tile([128, 128], f32, tag="tr")
                nc.tensor.transpose(pt[0:D, 0:128], e_blk[:, t, :], ident[:])
                nc.vector.tensor_copy(ebT[:, t * 128 : (t + 1) * 128], pt[0:D, 0:128])
            nc.vector.tensor_copy(ebT_bf[:], ebT[:].bitcast(f32))

            # ============ PHASE 2: scores + exp(elu) (Exp only) ============
            for t in range(MT):
                pt = psB.tile([128, 128], f32, tag="tr")
                nc.tensor.transpose(pt[0:D, 0:128], e_full[:, t, :], ident[:])
                eTt = work.tile([D, 128], f32r, tag="eTt", bufs=3)
                nc.vector.tensor_copy(eTt[:], pt[0:D, 0:128])
                gps = psA.tile([128, NBLK], f32, tag="big")
                nc.tensor.matmul(gps[:], eTt[:], ebT[:], start=True, stop=True)
                t_e = work.tile([128, NBLK], bf16, tag="elu_t", bufs=3)
                nc.scalar.activation(t_e[:], gps[:], AF.Exp)
                mn = work.tile([128, NBLK], bf16, tag="elu_m", bufs=3)
                nc.vector.tensor_scalar_min(mn[:], t_e[:], 1.0)
                v = work.tile([128, NBLK], bf16, tag="elu_v", bufs=3)
                nc.scalar.activation(v[:], mn[:], AF.Exp, bias=neg1_sb[:])
                nc.vector.scalar_tensor_tensor(
                    etn[t][:], t_e[:], 1.0, v[:], ALU.mult, ALU.max,
                    accum_out=cs_part[:, t : t + 1],
                )

            # colsum partials out; kick AllReduce
            nc.sync.dma_start(cs_in[:], cs_part[:])
            nc.gpsimd.collective_compute(
                "AllReduce", mybir.AluOpType.add, replica_groups=rg,
                ins=[cs_in[:]], outs=[cs_out[:]],
            )

            # ---- fill the AllReduce bubble: W^T formation (o-major) ----
            wpb_v = wpb_sb[:].rearrange("p (o q) -> p o q", q=KI + 1)
            for o in range(DOUT):
                wps = psA.tile([128, NBLK], f32, tag="big")
                nc.tensor.matmul(
                    wps[0 : KI + 1, :], wpb_v[:, o, :], ebT_bf[:],
                    start=True, stop=True,
                )
                if o % 2 == 0:
                    nc.vector.tensor_copy(wt_on[:, o, :], wps[0 : KI + 1, :])
                else:
                    nc.scalar.activation(wt_on[:, o, :], wps[0 : KI + 1, :], AF.Copy)

            # rcol = 1/colsum (blocks on AllReduce)
            nc.sync.dma_start(rcol[:], cs_out[:])
            nc.vector.reciprocal(rcol[:], rcol[:])
            for t in range(MT):
                nc.vector.tensor_scalar_mul(etn[t][:], etn[t][:], rcol[:, t : t + 1])

            # ============ PHASE 3: x_g1 = S_blk @ x ============
            for j in range(NBLK // 128):
                ps = psA.tile([128, BC], f32, tag="big")
                for t in range(MT):
                    nc.tensor.matmul(
                        ps[:],
                        etn[t][:, j * 128 : (j + 1) * 128],
                        x_sb[t][:],
                        start=(t == 0),
                        stop=(t == MT - 1),
                    )
                nc.vector.tensor_copy(xg1_bf[j][:], ps[:])
                nc.sync.dma_start(ag_in[j * 128 : (j + 1) * 128, :], xg1_bf[j][:])

            nc.gpsimd.collective_compute(
                "AllGather", mybir.AluOpType.bypass, replica_groups=rg,
                ins=[ag_in[:]], outs=[ag_out[:]],
            )

            # ---- fill the AllGather bubble: PE-transpose xg1 into xgT (k=1)
            for j in range(NBLK // 128):
                for ch in range(4):
                    tp = psBb.tile([128, 128], bf16, tag="trb")
                    nc.tensor.transpose(
                        tp[:], xg1_bf[j][:, ch * 128 : (ch + 1) * 128], identb[:]
                    )
                    for bl in range(4):
                        b = ch * 4 + bl
                        nc.vector.tensor_copy(
                            xgT_k1[:, j * 128 : (j + 1) * 128, b],
                            tp[bl * 32 : bl * 32 + 32, :],
                        )

            # reload gathered xg1 into x_sb (waits on AllGather)
            ago_re = ag_out.rearrange("(t p) f -> t p f", p=128)
            for t in range(MT):
                nc.sync.dma_start(x_sb[t][:], ago_re[t])

            # ===== PHASE 4: x_g2^T = 2*(S_blk @ xg1)^T - x^T, direct =====
            for g in range(4):
                ps = psA.tile([128, BC], f32, tag="big")
                for t in range(MT):
                    nc.tensor.matmul(
                        ps[:],
                        x_sb[t][:, g * 128 : (g + 1) * 128],
                        etn[t][:],
                        start=(t == 0),
                        stop=(t == MT - 1),
                    )
                for bl in range(4):
                    b = g * 4 + bl
                    nc.vector.scalar_tensor_tensor(
                        xgT_k2[:, :, b],
                        ps[bl * 32 : bl * 32 + 32, :],
                        2.0,
                        xgT_k0[:, :, b],
                        ALU.mult,
                        ALU.subtract,
                    )

        # ============ PHASE 5: per-node conv ============
        xgT_n = xgT[:].rearrange("p (n b) -> p n b", b=B)
        for grp in range(NBLK // 8):
            ps = psC.tile([16, 512], f32, tag="cps")
            for s in range(8):
                n = grp * 8 + s
                nc.tensor.matmul(
                    ps[0:B, s * DOUT : (s + 1) * DOUT],
                    xgT_n[:, n, :],
                    wt_on[:, :, n],
                    start=True,
                    stop=True,
                )
            q = grp % 2
            if q == 0:
                onat = outp.tile([16, 1024], f32, tag="onat")
            if grp % 2 == 0:
                nc.vector.tensor_copy(onat[:, q * 512 : (q + 1) * 512], ps[0:B, :])
            else:
                nc.scalar.activation(
                    onat[:, q * 512 : (q + 1) * 512], ps[0:B, :], AF.Copy
                )
            if q == 1:
                n0 = (grp - 1) * 8
                nc.sync.dma_start(out_d[:, n0 : n0 + 16, :], onat[:])

    nc.compile()
    return nc


def _get_program():
    if "nc" not in _CACHE:
        _CACHE["nc"] = _build_program()
    return _CACHE["nc"]


def _prepare_in_maps(x, node_embeddings, weights_pool, bias_pool, ln_gamma, ln_beta):
    import ml_dtypes

    bf16 = ml_dtypes.bfloat16
    x = np.asarray(x, dtype=np.float32)
    ne = np.asarray(node_embeddings, dtype=np.float32)
    wp = np.asarray(weights_pool, dtype=np.float32).reshape(D, CHEB_K * DIN, DOUT)
    bp = np.asarray(bias_pool, dtype=np.float32)
    gam = np.ascontiguousarray(np.asarray(ln_gamma, dtype=np.float32))
    bet = np.ascontiguousarray(np.asarray(ln_beta, dtype=np.float32))
    ident = np.eye(128, dtype=np.float32)
    identb = np.eye(128, dtype=np.float32).astype(bf16)

    # x transposed to [n, (b c)]
    xt = np.ascontiguousarray(x.transpose(1, 0, 2).reshape(N, BC).astype(bf16))
    # ne rearranged [(p), (t d)]
    ne_re = np.ascontiguousarray(
        ne.reshape(MT, 128, D).transpose(1, 0, 2).reshape(128, MT * D)
    )
    # weights_pool + bias packed: [d, o*(KI+1) + ki], bias at ki=KI
    wpb = np.zeros((D, DOUT * (KI + 1)), dtype=np.float32)
    for o in range(DOUT):
        wpb[:, o * (KI + 1) : o * (KI + 1) + KI] = wp[:, :, o]
        wpb[:, o * (KI + 1) + KI] = bp[:, o]
    wpb = wpb.astype(bf16)

    in_maps = []
    for c in range(NCORES):
        sl = slice(c * NBLK, (c + 1) * NBLK)
        # x^T own block: [c, (n, b)]
        xTb = np.ascontiguousarray(
            x[:, sl, :].transpose(2, 1, 0).reshape(DIN, NBLK * B).astype(bf16)
        )
        neb_re = np.ascontiguousarray(
            ne[sl].reshape(NBLK // 128, 128, D).transpose(1, 0, 2)
            .reshape(128, (NBLK // 128) * D)
        )
        in_maps.append(
            {
                "x_t": xt,
                "xTb": xTb,
                "ne_re": ne_re,
                "neb_re": neb_re,
                "wpb": wpb,
                "gam": gam,
                "bet": bet,
                "ident": ident,
                "identb": identb,
            }
        )
    return in_maps


def kernel(x, node_embeddings, weights_pool, bias_pool, ln_gamma, ln_beta):
    from concourse.bass_utils import run_bass_kernel_spmd

    nc = _get_program()
    in_maps = _prepare_in_maps(
        x, node_embeddings, weights_pool, bias_pool, ln_gamma, ln_beta
    )
    res = run_bass_kernel_spmd(nc, in_maps, list(range(NCORES)))
    out = np.concatenate([res.results[c]["out_blk"] for c in range(NCORES)], axis=1)
    return out
